# revision 1
# baseline (speedup 1.0000x reference)
"""v2 Bass/Trainium2 kernel for nn_DeepLSTMNet: wavefront LSTM.

Same data-parallel strategy as v1 (8 cores x 512 batch rows, transposed
[H, B] state layout, bias as a ones-row on the recurrent matmul, fp16
matmuls). Improvements over v1:

- Wavefront schedule: at wave w, layer 1 processes step w, layer 2 step
  w-1, layer 3 step w-2, layer 4 step w-3. This makes L3 and L4 cells
  independent within a wave, so their gates share one PSUM tile (L3 at
  partitions [0:48), L4 at [64:88)) and one set of merged
  activation/vector instructions -- the ScalarE/VectorE instruction
  count per step drops ~25-30%.
- In-place DVE ops (c *= f; tg *= i; c += tg) -- no temp tiles.
- h2/h3/h4 products offloaded to GPSIMD; h1 stays on VectorE.
"""

import numpy as np

IN_DIM = 64
HS = [90, 66, 48, 24]
OUT_DIM = 1
T_TOTAL = 512
B_TOTAL = 4096
N_CORES = 8
BL = B_TOTAL // N_CORES

H4_BASE = 64  # partition base for layer-4 state/gates (matmul dst limit: {0,32,64})


def _gate_perm(h):
    return np.r_[0:h, h : 2 * h, 3 * h : 4 * h, 2 * h : 3 * h]


def prep_weights(inputs):
    out = {}
    dins = [IN_DIM] + HS[:-1]
    for l in range(4):
        h = HS[l]
        perm = _gate_perm(h)
        wih = np.asarray(inputs[f"Wih{l + 1}"], dtype=np.float32)[perm]
        whh = np.asarray(inputs[f"Whh{l + 1}"], dtype=np.float32)[perm]
        b = (
            np.asarray(inputs[f"bih{l + 1}"], dtype=np.float32)
            + np.asarray(inputs[f"bhh{l + 1}"], dtype=np.float32)
        )[perm]
        out[f"win{l}"] = np.ascontiguousarray(wih.T)
        out[f"wrec{l}"] = np.ascontiguousarray(np.vstack([whh.T, b[None, :]]))
        assert out[f"win{l}"].shape == (dins[l], 4 * h)
    # fuse layer-4's input+recurrent into one K=89 lhsT matching the S34
    # state layout [h3(48); 1; pad(15); h4(24); 1]
    w4 = np.zeros((89, 4 * HS[3]), dtype=np.float32)
    win4 = out.pop("win3")   # [48, 96]
    wrec4 = out.pop("wrec3")  # [25, 96]
    w4[0:48] = win4
    w4[48] = wrec4[24]  # bias row
    w4[64:88] = wrec4[0:24]
    out["w4f"] = w4
    # pad layer-3 weights per-gate to 64 output cols so its matmuls also
    # write the [48:64) PSUM rows shared with layer 4 (keeps them zero)
    h = HS[2]
    for nm in ("win2", "wrec2"):
        w = out[nm]
        wp = np.zeros((w.shape[0], 4 * 64), dtype=np.float32)
        for g in range(4):
            wp[:, 64 * g : 64 * g + h] = w[:, h * g : h * (g + 1)]
        out[nm] = wp
    fc_w = np.asarray(inputs["fc_w"], dtype=np.float32)
    fc_b = np.asarray(inputs["fc_b"], dtype=np.float32)
    out["wfc"] = np.ascontiguousarray(np.vstack([fc_w.T, fc_b[None, :]]))
    return {k: v.astype(np.float16) for k, v in out.items()}


def prep_x_core(x, core, t_total=T_TOTAL):
    xs = np.asarray(x, dtype=np.float32)[core * BL : (core + 1) * BL, :t_total, :]
    return np.ascontiguousarray(xs.transpose(2, 1, 0)).astype(np.float16)  # [D, T, BL]


def build_program(t_total=T_TOTAL, u=64):
    import concourse.bass as bass
    import concourse.tile as tile
    from concourse import bacc, mybir
    from concourse.bass import ds

    assert t_total >= 4
    f32 = mybir.dt.float32
    f16 = mybir.dt.float16
    AF = mybir.ActivationFunctionType
    OP = mybir.AluOpType
    dins = [IN_DIM] + HS[:-1]
    H1, H2, H3, H4 = HS
    B4 = H4_BASE

    nc = bacc.Bacc("TRN2", target_bir_lowering=False, debug=False)

    x_dram = nc.dram_tensor("xT", [IN_DIM, t_total, BL], f16, kind="ExternalInput")
    win_dram = []
    wrec_dram = []
    gw = [4 * HS[0], 4 * HS[1], 4 * 64, 4 * HS[3]]
    for l in range(3):
        win_dram.append(
            nc.dram_tensor(f"win{l}", [dins[l], gw[l]], f16, kind="ExternalInput")
        )
        wrec_dram.append(
            nc.dram_tensor(f"wrec{l}", [HS[l] + 1, gw[l]], f16, kind="ExternalInput")
        )
    w4f_dram = nc.dram_tensor("w4f", [89, 4 * HS[3]], f16, kind="ExternalInput")
    wfc_dram = nc.dram_tensor("wfc", [HS[3] + 1, OUT_DIM], f16, kind="ExternalInput")
    ones_dram = nc.dram_tensor("ones", [1, BL], f16, kind="ExternalInput")
    zeros_dram = nc.dram_tensor("zeros", [128, BL], f16, kind="ExternalInput")
    out_dram = nc.dram_tensor("out", [OUT_DIM, BL], f32, kind="ExternalOutput")

    with tile.TileContext(nc) as tc:
        with (
            tc.tile_pool(name="const", bufs=1) as const,
            tc.tile_pool(name="state", bufs=1) as state,
            tc.tile_pool(name="xp", bufs=2) as xp,
            tc.tile_pool(name="actp", bufs=2) as actp,
            tc.tile_pool(name="psp", bufs=2, space="PSUM") as psp,
        ):
            # --- weights ---
            win_t = []
            wrec_t = []
            for l in range(3):
                wt = const.tile([dins[l], gw[l]], f16, tag=f"win{l}")
                nc.sync.dma_start(out=wt, in_=win_dram[l][:, :])
                win_t.append(wt)
                wr = const.tile([HS[l] + 1, gw[l]], f16, tag=f"wrec{l}")
                nc.sync.dma_start(out=wr, in_=wrec_dram[l][:, :])
                wrec_t.append(wr)
            # layer 4: single fused lhsT [89, 96] matching S34 layout
            w4f = const.tile([89, 4 * H4], f16, tag="w4f")
            nc.sync.dma_start(out=w4f, in_=w4f_dram[:, :])
            # fc weights placed at [64:89) (rhs = S34[64:89))
            wfc_t = const.tile([128, OUT_DIM], f16, tag="wfc")
            nc.sync.dma_start(out=wfc_t[B4 : B4 + H4 + 1, :], in_=wfc_dram[:, :])
            # constants for psum zeroing
            zrow = const.tile([1, 128], f16, tag="zrow")
            nc.sync.dma_start(out=zrow, in_=zeros_dram[0:1, 0:128])
            onesr = const.tile([1, BL], f16, tag="onesr")
            nc.sync.dma_start(out=onesr, in_=ones_dram[:, :])

            # --- states ---
            # S1 [91]: h1 + ones ; S2 [67] ; S3 [49] ; S4 [89]: h4 at
            # [64:88), ones at 88. C34 [88]: c3 [0:48), c4 [64:88).
            S1 = state.tile([H1 + 1, BL], f16, tag="S1")
            nc.sync.dma_start(out=S1[0:H1, :], in_=zeros_dram[0:H1, :])
            nc.sync.dma_start(out=S1[H1 : H1 + 1, :], in_=ones_dram[:, :])
            S2 = state.tile([H2 + 1, BL], f16, tag="S2")
            nc.sync.dma_start(out=S2[0:H2, :], in_=zeros_dram[0:H2, :])
            nc.sync.dma_start(out=S2[H2 : H2 + 1, :], in_=ones_dram[:, :])
            # S34 [89]: h3 [0:48), ones@48, pad, h4 [64:88), ones@88
            S34 = state.tile([89, BL], f16, tag="S34")
            nc.sync.dma_start(out=S34, in_=zeros_dram[0:89, :])
            nc.sync.dma_start(out=S34[H3 : H3 + 1, :], in_=ones_dram[:, :])
            nc.sync.dma_start(out=S34[B4 + H4 : B4 + H4 + 1, :], in_=ones_dram[:, :])
            C1 = state.tile([H1, BL], f32, tag="C1")
            nc.vector.memset(C1, 0.0)
            C2 = state.tile([H2, BL], f32, tag="C2")
            nc.vector.memset(C2, 0.0)
            C34 = state.tile([B4 + H4, BL], f32, tag="C34")
            nc.vector.memset(C34, 0.0)

            # --- zero both PSUM slots once so never-written partition rows
            # stay finite (0) forever; sig(0)=0.5 updates on inactive-layer
            # regions are mathematically harmless no-ops on zero states.
            for _ in range(2):
                gz = psp.tile([128, 4, 512], f32, tag="G")
                for b in range(4):
                    nc.tensor.matmul(
                        gz[:, b, :], zrow, onesr, start=True, stop=True
                    )

            P34 = B4 + H4  # 88

            def mm_l1(x_ap, g):
                # all input MMs first, then all recurrent MMs: consecutive
                # matmuls hit different PSUM banks so drains overlap fills
                for gi in range(4):
                    gs = slice(gi * H1, (gi + 1) * H1)
                    nc.tensor.matmul(
                        g[0:H1, gi, :], win_t[0][:, gs], x_ap, start=True, stop=False
                    )
                for gi in range(4):
                    gs = slice(gi * H1, (gi + 1) * H1)
                    nc.tensor.matmul(
                        g[0:H1, gi, :],
                        wrec_t[0][:, gs],
                        S1[0 : H1 + 1, :],
                        start=False,
                        stop=True,
                    )

            def mm_l4(g):
                for gi in range(4):
                    gs = slice(gi * H4, (gi + 1) * H4)
                    nc.tensor.matmul(
                        g[B4:P34, gi, :],
                        w4f[:, gs],
                        S34[0:89, :],
                        start=True,
                        stop=True,
                        skip_group_check=True,
                        tile_position=(0, B4),
                    )

            def mm_mid(l, g):
                # l = 1 or 2 (layers 2 and 3); L3 writes 64-wide padded gates
                h = HS[l]
                m = 64 if l == 2 else h
                for gi in range(4):
                    gs = slice(gi * m, gi * m + m)
                    nc.tensor.matmul(
                        g[0:m, gi, :],
                        win_t[l][:, gs],
                        (S1 if l == 1 else S2)[0 : HS[l - 1], :],
                        start=True,
                        stop=False,
                    )
                for gi in range(4):
                    gs = slice(gi * m, gi * m + m)
                    nc.tensor.matmul(
                        g[0:m, gi, :],
                        wrec_t[l][:, gs],
                        (S2 if l == 1 else S34)[0 : h + 1, :],
                        start=False,
                        stop=True,
                    )

            def cell34(g):
                sig = actp.tile([P34, 3, BL], f32, tag="sig34")
                tg = actp.tile([P34, BL], f32, tag="tg34")
                tcn = actp.tile([P34, BL], f32, tag="tc34")
                nc.scalar.activation(sig, g[0:P34, 0:3, :], AF.Sigmoid)
                nc.scalar.activation(tg, g[0:P34, 3, :], AF.Tanh)
                nc.vector.tensor_tensor(C34, C34, sig[:, 1, :], OP.mult)
                nc.vector.tensor_tensor(tg, sig[:, 0, :], tg, OP.mult)
                nc.vector.tensor_tensor(C34, C34, tg, OP.add)
                nc.scalar.activation(tcn, C34, AF.Tanh)
                nc.gpsimd.tensor_tensor(
                    S34[0:H3, :], sig[0:H3, 2, :], tcn[0:H3, :], OP.mult
                )
                nc.gpsimd.tensor_tensor(
                    S34[B4:P34, :], sig[B4:P34, 2, :], tcn[B4:P34, :], OP.mult
                )

            def cell_lo(l, g):
                # l = 0 (layer 1) or 1 (layer 2)
                h = HS[l]
                sig = actp.tile([h, 3, BL], f32, tag=f"sig{l}")
                tg = actp.tile([h, BL], f32, tag=f"tg{l}")
                tcn = actp.tile([h, BL], f32, tag=f"tc{l}")
                C = C1 if l == 0 else C2
                S = S1 if l == 0 else S2
                nc.scalar.activation(sig, g[0:h, 0:3, :], AF.Sigmoid)
                nc.scalar.activation(tg, g[0:h, 3, :], AF.Tanh)
                nc.vector.tensor_tensor(C, C, sig[:, 1, :], OP.mult)
                nc.vector.tensor_tensor(tg, sig[:, 0, :], tg, OP.mult)
                nc.vector.tensor_tensor(C, C, tg, OP.add)
                nc.scalar.activation(tcn, C, AF.Tanh)
                if l == 0:
                    nc.vector.tensor_tensor(S[0:h, :], sig[:, 2, :], tcn, OP.mult)
                else:
                    nc.gpsimd.tensor_tensor(S[0:h, :], sig[:, 2, :], tcn, OP.mult)

            def cell3_only(g):
                sig = actp.tile([P34, 3, BL], f32, tag="sig34", name="sig3o")
                tg = actp.tile([P34, BL], f32, tag="tg34", name="tg3o")
                tcn = actp.tile([P34, BL], f32, tag="tc34", name="tc3o")
                nc.scalar.activation(sig[0:H3], g[0:H3, 0:3, :], AF.Sigmoid)
                nc.scalar.activation(tg[0:H3], g[0:H3, 3, :], AF.Tanh)
                nc.vector.tensor_tensor(C34[0:H3], C34[0:H3], sig[0:H3, 1, :], OP.mult)
                nc.vector.tensor_tensor(tg[0:H3], sig[0:H3, 0, :], tg[0:H3], OP.mult)
                nc.vector.tensor_tensor(C34[0:H3], C34[0:H3], tg[0:H3], OP.add)
                nc.scalar.activation(tcn[0:H3], C34[0:H3], AF.Tanh)
                nc.gpsimd.tensor_tensor(
                    S34[0:H3, :], sig[0:H3, 2, :], tcn[0:H3], OP.mult
                )

            def cell4_only(g):
                sig = actp.tile([P34, 3, BL], f32, tag="sig34", name="sig4o")
                tg = actp.tile([P34, BL], f32, tag="tg34", name="tg4o")
                tcn = actp.tile([P34, BL], f32, tag="tc34", name="tc4o")
                nc.scalar.activation(sig[B4:P34], g[B4:P34, 0:3, :], AF.Sigmoid)
                nc.scalar.activation(tg[B4:P34], g[B4:P34, 3, :], AF.Tanh)
                nc.vector.tensor_tensor(
                    C34[B4:P34], C34[B4:P34], sig[B4:P34, 1, :], OP.mult
                )
                nc.vector.tensor_tensor(
                    tg[B4:P34], sig[B4:P34, 0, :], tg[B4:P34], OP.mult
                )
                nc.vector.tensor_tensor(C34[B4:P34], C34[B4:P34], tg[B4:P34], OP.add)
                nc.scalar.activation(tcn[B4:P34], C34[B4:P34], AF.Tanh)
                nc.gpsimd.tensor_tensor(
                    S34[B4:P34, :], sig[B4:P34, 2, :], tcn[B4:P34], OP.mult
                )

            def wave(x_ap, a1, a2, a3, a4):
                g1 = psp.tile([128, 4, 512], f32, tag="G", name="g1") if a1 else None
                g2 = psp.tile([128, 4, 512], f32, tag="G", name="g2") if a2 else None
                g34 = psp.tile([128, 4, 512], f32, tag="G", name="g34") if (a3 or a4) else None
                # MM phase: all reads of old state happen before cell writes
                if a1:
                    mm_l1(x_ap, g1)
                if a2:
                    mm_mid(1, g2)
                if a3:
                    mm_mid(2, g34)
                if a4:
                    mm_l4(g34)
                # cell phase (inactive-layer regions see zero/stale finite
                # gates; the resulting state updates only touch dead state)
                if a1:
                    cell_lo(0, g1)
                if a2:
                    cell_lo(1, g2)
                if a3 and a4:
                    cell34(g34)
                elif a3:
                    cell3_only(g34)
                elif a4:
                    cell4_only(g34)

            n_main = max(0, (t_total - 3) // u)
            main_lo, main_hi = 3, 3 + n_main * u

            # prologue waves 0..2
            pro_len = min(u, t_total)
            xpro = xp.tile([IN_DIM, u, BL], f16, tag="xt")
            nc.sync.dma_start(out=xpro[:, 0:pro_len, :], in_=x_dram[:, 0:pro_len, :])
            wave(xpro[:, 0, :], True, False, False, False)
            wave(xpro[:, 1, :], True, True, False, False)
            wave(xpro[:, 2, :], True, True, True, False)

            # main loop
            if n_main > 0:
                with tc.For_i(main_lo, main_hi, u) as it:
                    xt = xp.tile([IN_DIM, u, BL], f16, tag="xt")
                    nc.sync.dma_start(out=xt, in_=x_dram[:, ds(it, u), :])
                    for uu in range(u):
                        wave(xt[:, uu, :], True, True, True, True)

            # leftover full waves [main_hi, t_total)
            if main_hi < t_total:
                ep_len = t_total - (t_total - u if t_total >= u else 0)
                ep_base = t_total - ep_len
                xep = xp.tile([IN_DIM, u, BL], f16, tag="xt")
                nc.sync.dma_start(
                    out=xep[:, 0:ep_len, :], in_=x_dram[:, ep_base:t_total, :]
                )
                for w in range(main_hi, t_total):
                    wave(xep[:, w - ep_base, :], True, True, True, True)

            # winding-down partial waves
            wave(None, False, True, True, True)
            wave(None, False, False, True, True)
            wave(None, False, False, False, True)

            # --- fc ---
            fcp = psp.tile([128, 4, 512], f32, tag="G")
            nc.tensor.matmul(
                fcp[0:OUT_DIM, 0, :],
                wfc_t[B4 : B4 + H4 + 1, :],
                S34[B4 : B4 + H4 + 1, :],
                start=True,
                stop=True,
                skip_group_check=True,
                tile_position=(B4, 0),
            )
            ot = const.tile([OUT_DIM, BL], f32, tag="ot")
            nc.vector.tensor_copy(ot, fcp[0:OUT_DIM, 0, :])
            nc.sync.dma_start(out=out_dram[:, :], in_=ot)

    nc.compile()
    return nc


def run(inputs, t_total=T_TOTAL, u=64, trace=False, **spmd_kwargs):
    from concourse import bass_utils

    nc = build_program(t_total=t_total, u=u)
    w = prep_weights(inputs)
    w["ones"] = np.ones((1, BL), dtype=np.float16)
    w["zeros"] = np.zeros((128, BL), dtype=np.float16)
    in_maps = []
    for core in range(N_CORES):
        m = dict(w)
        m["xT"] = prep_x_core(inputs["x"], core, t_total)
        in_maps.append(m)
    res = bass_utils.run_bass_kernel_spmd(
        nc, in_maps, core_ids=list(range(N_CORES)), trace=trace, **spmd_kwargs
    )
    out = np.empty((B_TOTAL, OUT_DIM), dtype=np.float32)
    for core in range(N_CORES):
        out[core * BL : (core + 1) * BL, 0] = res.results[core]["out"][0]
    return out, res


def kernel(**inputs):
    out, _ = run(inputs)
    return out

